# revision 14
# baseline (speedup 1.0000x reference)
"""Complex-valued multihead attention (B=4, T=1024, C=512, H=8) on 8 Trainium2
NeuronCores.

Sharding: core c = (b = c//2, half = c%2) handles batch b and heads
half*4 .. half*4+3 (tensor-parallel over heads within a batch).  The out_proj
is computed as per-core partials over each core's 256 head-dims and summed on
the host (the reduce of the head-TP all-gather), where the bo bias is added.

On-device math uses float32r (full-rate fp32 PE path, ~1e-4 component error).
Complex matmuls are computed as single K=128 matmuls by stacking real/imag
parts along the contraction axis, e.g.
  Sr = [Qr;Qi]^T-stationary x [Kr;-Ki]-moving  (t x s tile in one matmul).
Linear-layer biases are folded in as K=1 fp16 matmuls accumulating into PSUM.
"""
import os

import numpy as np

B, T, C, H = 4, 1024, 512, 8
D = C // H  # 64
O = 256     # head-dims per core (4 heads x 64)
SCALE = D ** (-0.5)
N_CORES = 8
IC = C // 128  # 4 contraction chunks over c_in
TP = T // 128  # 8 t-partition chunks
SF = T // 512  # 2 free-dim chunks of 512

_prog_cache = {}
last_exec_ns = None
last_scope_times = None


def _build_program():
    import concourse.bacc as bacc
    import concourse.tile as tile
    from concourse import mybir

    F32 = mybir.dt.float32
    F32R = mybir.dt.float32r
    F16 = mybir.dt.float16
    BF16 = mybir.dt.bfloat16
    AF = mybir.ActivationFunctionType
    ALU = mybir.AluOpType

    nc = bacc.Bacc("TRN2", target_bir_lowering=False, debug=False,
                   num_devices=N_CORES)

    # ---- DRAM I/O ----
    # xt: ic-major free-dim packing, one DMA each: [128, IC*T]
    xt_r = nc.dram_tensor("xt_r", [128, IC * T], F32R, kind="ExternalInput").ap()
    xt_i = nc.dram_tensor("xt_i", [128, IC * T], F32R, kind="ExternalInput").ap()
    # Q/K weights: per (head, pass): [128, 1024] = [a ic0..3 | b ic0..3]
    wqk = nc.dram_tensor("wqk", [4, 2, 128, 1024], F32R,
                         kind="ExternalInput").ap()
    # V weights: [2, 128, IC*512] = (wv1_a, wv1_b), ic-major packing
    wv_pack = nc.dram_tensor("wv_pack", [2, 128, IC * 512], F32R,
                             kind="ExternalInput").ap()
    # out-proj: [128, 4096] = [wo_r heads 0..3 | wo_i heads 0..3]
    wo_pack = nc.dram_tensor("wo_pack", [128, 4096], F32R,
                             kind="ExternalInput").ap()
    # col blocks: [0:512]=ones, [512:1024]=bias_va, [1024:1536]=bias_vb,
    # then 8 blocks of 128: bq[lh]*4, bk1[lh]*4
    bias_pack = nc.dram_tensor("bias_pack", [1, 2560], F16,
                               kind="ExternalInput").ap()

    attw_r = nc.dram_tensor("attw_r", [4, T, T], F32, kind="ExternalOutput").ap()
    attw_i = nc.dram_tensor("attw_i", [4, T, T], F32, kind="ExternalOutput").ap()
    yp_r = nc.dram_tensor("yp_r", [T, C], F32, kind="ExternalOutput").ap()
    yp_i = nc.dram_tensor("yp_i", [T, C], F32, kind="ExternalOutput").ap()

    with tile.TileContext(nc) as tc:
        with (
            tc.tile_pool(name="xt", bufs=1) as xt_pool,
            tc.tile_pool(name="wt", bufs=2) as wt_pool,
            tc.tile_pool(name="wv", bufs=1) as wv_pool,
            tc.tile_pool(name="wo", bufs=1) as wo_pool,
            tc.tile_pool(name="qk", bufs=2) as qk_pool,
            tc.tile_pool(name="vs", bufs=1) as vs_pool,
            tc.tile_pool(name="st", bufs=16) as st_pool,
            tc.tile_pool(name="sdr", bufs=3) as sdr_pool,
            tc.tile_pool(name="av", bufs=1) as av_pool,
            tc.tile_pool(name="yp", bufs=2) as yp_pool,
            tc.tile_pool(name="cst", bufs=1) as cst_pool,
            tc.tile_pool(name="pp", bufs=2, space="PSUM") as pp_pool,
            tc.tile_pool(name="psc", bufs=4, space="PSUM") as psc_pool,
            tc.tile_pool(name="pav", bufs=2, space="PSUM") as pav_pool,
        ):
            # ---- inputs: large DMAs split in halves across three queues,
            # ordered so the V-projection's operands land first ----
            xtr_all = xt_pool.tile([128, IC * T], F32R, tag="xtr")
            half_x = IC * T // 2
            nc.sync.dma_start(xtr_all[:, 0:half_x], xt_r[:, 0:half_x])
            xti_all = xt_pool.tile([128, IC * T], F32R, tag="xti")
            nc.gpsimd.dma_start(xti_all[:, 0:half_x], xt_i[:, 0:half_x])
            wv_sb = []
            for v in range(2):
                t = wv_pool.tile([128, IC * 512], F32R, tag=f"wv{v}")
                nc.scalar.dma_start(t[:], wv_pack[v])
                wv_sb.append(t)
            nc.sync.dma_start(xtr_all[:, half_x:], xt_r[:, half_x:])
            nc.gpsimd.dma_start(xti_all[:, half_x:], xt_i[:, half_x:])
            xtr = [xtr_all[:, ic * T:(ic + 1) * T] for ic in range(IC)]
            xti = [xti_all[:, ic * T:(ic + 1) * T] for ic in range(IC)]

            bias_all = cst_pool.tile([1, 2560], F16, tag="bias_all")
            nc.scalar.dma_start(bias_all[:], bias_pack[:])
            ones_sb = bias_all[:, 0:512]
            bva_sb = bias_all[:, 512:1024]
            bvb_sb = bias_all[:, 1024:1536]
            bias_sb = {}
            for j, nm in enumerate(("bq", "bk1")):
                for lh in range(4):
                    off = 1536 + (j * 4 + lh) * 128
                    bias_sb[(nm, lh)] = bias_all[:, off:off + 128]

            wo_all = wo_pool.tile([128, 4096], F32R, tag="wo")
            nc.scalar.dma_start(wo_all[:], wo_pack[:])
            wo_sb = {}
            for lh in range(4):
                wo_sb[("r", lh)] = wo_all[:, lh * 512:(lh + 1) * 512]
                wo_sb[("i", lh)] = wo_all[:, 2048 + lh * 512:2048 + (lh + 1) * 512]

            # ---- vstk_a projection (all heads batched); vstk_b derived ----
            # vstk_a = [Vr_h|Vi_h]x4 heads; vstk_b = [-Vi_h|Vr_h]x4 via ACT
            # column swap + negate from vstk_a.
            vstk = {}
            scope_vproj = nc.named_scope("vproj")
            scope_vproj.__enter__()
            for tp in range(TP):
                ps = pp_pool.tile([128, 512], F32, tag="pp")
                for ic in range(IC):
                    nc.tensor.matmul(
                        ps[:], xtr[ic][:, tp * 128:(tp + 1) * 128],
                        wv_sb[0][:, ic * 512:(ic + 1) * 512],
                        start=(ic == 0), stop=False)
                    nc.tensor.matmul(
                        ps[:], xti[ic][:, tp * 128:(tp + 1) * 128],
                        wv_sb[1][:, ic * 512:(ic + 1) * 512],
                        start=False, stop=False)
                nc.tensor.matmul(
                    ps[:], ones_sb[:, 0:128], bva_sb,
                    start=False, stop=True)
                dst = vs_pool.tile([128, 512], F32R, tag=f"va_{tp}")
                nc.scalar.activation(dst[:], ps[:], AF.Identity,
                                     bias=0.0, scale=1.0)
                vstk[("a", tp)] = dst
                dst_b = vs_pool.tile([128, 512], F32R, tag=f"vb_{tp}")
                for lh in range(4):
                    o = lh * 128
                    nc.scalar.activation(dst_b[:, o:o + 64],
                                         dst[:, o + 64:o + 128],
                                         AF.Identity, bias=0.0, scale=-1.0)
                    nc.scalar.activation(dst_b[:, o + 64:o + 128],
                                         dst[:, o:o + 64],
                                         AF.Identity, bias=0.0, scale=1.0)
                vstk[("b", tp)] = dst_b

            scope_vproj.__exit__(None, None, None)
            avstk = {}
            qk_state = {}

            def emit_proj(lh, name, bnm, pidx):
                w_t = wt_pool.tile([128, 1024], F32R, tag="w")
                (nc.sync if pidx == 0 else nc.gpsimd).dma_start(
                    w_t[:], wqk[lh, pidx])
                dst = qk_pool.tile([128, T], F32R, tag=name)
                for tf in range(SF):
                    ps = pp_pool.tile([128, 512], F32, tag="pp")
                    for ic in range(IC):
                        nc.tensor.matmul(
                            ps[:], w_t[:, ic * 128:(ic + 1) * 128],
                            xtr[ic][:, tf * 512:(tf + 1) * 512],
                            start=(ic == 0), stop=False)
                        nc.tensor.matmul(
                            ps[:], w_t[:, 512 + ic * 128:512 + (ic + 1) * 128],
                            xti[ic][:, tf * 512:(tf + 1) * 512],
                            start=False, stop=False)
                    nc.tensor.matmul(
                        ps[:], bias_sb[(bnm, lh)], ones_sb,
                        start=False, stop=True)
                    nc.scalar.activation(dst[:, tf * 512:(tf + 1) * 512],
                                         ps[:], AF.Identity,
                                         bias=0.0, scale=1.0)
                qk_state[(lh, name)] = dst

            def emit_kts(lh):
                ktn = qk_state[(lh, "ktn")]
                kts = qk_pool.tile([128, T], F32R, tag="kts")
                nc.gpsimd.dma_start(kts[0:64, :], ktn[64:128, :])
                nc.sync.dma_start(kts[64:128, :], ktn[0:64, :])
                nc.scalar.activation(kts[0:64, :], kts[0:64, :], AF.Identity,
                                     bias=0.0, scale=-1.0)
                qk_state[(lh, "kts")] = kts

            def emit_scores(lh, th, part, stt):
                qt = qk_state[(lh, "qt")]
                kmat = qk_state[(lh, "ktn" if part == "r" else "kts")]
                for k in range(8):
                    sp = k
                    ps_st = psc_pool.tile([128, 512], F32, tag="psc")
                    nc.tensor.matmul(
                        ps_st[:], kmat[:, sp * 128:(sp + 1) * 128],
                        qt[:, th * 512:(th + 1) * 512],
                        start=True, stop=True)
                    st_t = st_pool.tile([128, 512], F32R, tag="st")
                    nc.scalar.activation(st_t[:], ps_st[:], AF.Relu,
                                         bias=0.0, scale=SCALE)
                    stt[(part, sp)] = st_t

                    tp = th * 4 + k // 2
                    sf = k % 2
                    ps_s = psc_pool.tile([128, 512], F32, tag="psc")
                    nc.tensor.matmul(
                        ps_s[:], qt[:, tp * 128:(tp + 1) * 128],
                        kmat[:, sf * 512:(sf + 1) * 512],
                        start=True, stop=True)
                    s_t = sdr_pool.tile([128, 512], F32, tag="sdr")
                    nc.vector.tensor_scalar(s_t[:], ps_s[:], SCALE,
                                            0.0, ALU.mult, ALU.max)
                    attw = attw_r if part == "r" else attw_i
                    eng = nc.sync if part == "r" else nc.gpsimd
                    eng.dma_start(
                        attw[lh, tp * 128:(tp + 1) * 128,
                             sf * 512:(sf + 1) * 512], s_t[:])

            def emit_av(lh, th, stt):
                ps_av = pav_pool.tile([128, 512], F32, tag="pav")
                for sp in range(8):
                    nc.tensor.matmul(
                        ps_av[:],
                        vstk[("a", sp)][:, lh * 128:(lh + 1) * 128],
                        stt[("r", sp)][:],
                        start=(sp == 0), stop=False)
                    nc.tensor.matmul(
                        ps_av[:],
                        vstk[("b", sp)][:, lh * 128:(lh + 1) * 128],
                        stt[("i", sp)][:],
                        start=False, stop=(sp == 7))
                if th == 0:
                    av_sb = av_pool.tile([128, T], F32R, tag=f"av{lh}")
                    avstk[lh] = av_sb
                nc.scalar.activation(
                    avstk[lh][:, th * 512:(th + 1) * 512], ps_av[:],
                    AF.Identity, bias=0.0, scale=1.0)

            def emit_yp(tp):
                ps_r = pp_pool.tile([128, 512], F32, tag="pp")
                ps_i = pp_pool.tile([128, 512], F32, tag="pp")
                for lh in range(4):
                    nc.tensor.matmul(ps_r[:],
                                     avstk[lh][:, tp * 128:(tp + 1) * 128],
                                     wo_sb[("r", lh)][:],
                                     start=(lh == 0), stop=(lh == 3))
                for lh in range(4):
                    nc.tensor.matmul(ps_i[:],
                                     avstk[lh][:, tp * 128:(tp + 1) * 128],
                                     wo_sb[("i", lh)][:],
                                     start=(lh == 0), stop=(lh == 3))
                o_r = yp_pool.tile([128, 512], F32, tag="yp")
                nc.vector.tensor_copy(o_r[:], ps_r[:])
                nc.sync.dma_start(yp_r[tp * 128:(tp + 1) * 128, :], o_r[:])
                o_i = yp_pool.tile([128, 512], F32, tag="yp")
                nc.vector.tensor_copy(o_i[:], ps_i[:])
                nc.gpsimd.dma_start(yp_i[tp * 128:(tp + 1) * 128, :], o_i[:])

            # prologue: head0 projections
            emit_proj(0, "qt", "bq", 0)
            emit_proj(0, "ktn", "bk1", 1)
            emit_kts(0)
            for lh in range(4):
                scope_h = nc.named_scope(f"head{lh}")
                scope_h.__enter__()
                stt0 = {}
                emit_scores(lh, 0, "r", stt0)
                emit_scores(lh, 0, "i", stt0)
                if lh < 3:
                    emit_proj(lh + 1, "qt", "bq", 0)
                emit_av(lh, 0, stt0)
                if lh < 3:
                    emit_proj(lh + 1, "ktn", "bk1", 1)
                    emit_kts(lh + 1)
                stt1 = {}
                emit_scores(lh, 1, "r", stt1)
                emit_scores(lh, 1, "i", stt1)
                if lh == 3:
                    for tp in range(4):
                        emit_yp(tp)
                emit_av(lh, 1, stt1)
                scope_h.__exit__(None, None, None)
            # ---- out_proj partials ----
            scope_yp = nc.named_scope("yp")
            scope_yp.__enter__()
            for tp in range(4, TP):
                emit_yp(tp)

            scope_yp.__exit__(None, None, None)
    nc.compile()
    return nc


def _head_stacks(Wr, Wi, hs):
    # -> per-head (512, 128) transposed stationary blocks
    return Wr[hs, :].T.copy(), Wi[hs, :].T.copy()


def _as_chunks(m):
    # (512, 128) -> (IC, 128, 128)
    return np.ascontiguousarray(m.reshape(IC, 128, 128))


def _pack_ic(m):
    # (512, N) -> (128, IC*N): ic-major packing of row chunks into free dim
    n = m.shape[1]
    return np.ascontiguousarray(
        m.reshape(IC, 128, n).transpose(1, 0, 2).reshape(128, IC * n))


def _core_inputs(query, Wq, bq, Wk, bk, Wv, bv, Wo, bo, b, half):
    f32 = np.float32
    x = query[b]
    xt_r = _pack_ic(np.ascontiguousarray(x.real.T).astype(f32))
    xt_i = _pack_ic(np.ascontiguousarray(x.imag.T).astype(f32))

    WqT_r, WqT_i = Wq.real.T.astype(f32), Wq.imag.T.astype(f32)
    WkT_r, WkT_i = Wk.real.T.astype(f32), Wk.imag.T.astype(f32)
    WvT_r, WvT_i = Wv.real.T.astype(f32), Wv.imag.T.astype(f32)
    WoT_r, WoT_i = Wo.real.T.astype(f32), Wo.imag.T.astype(f32)

    def headcols(WT_r, WT_i, pair):
        # (512, 128) per head: [left 64 | right 64] col blocks, then all
        # heads side by side -> (512, 512)
        src_ = {"r": WT_r, "i": WT_i}
        cols = []
        for lh in range(4):
            g = half * 4 + lh
            hs = slice(g * D, (g + 1) * D)
            cols.append(np.concatenate(
                [s * src_[k][:, hs] for s, k in pair], axis=1))
        return np.concatenate(cols, axis=1)

    # wqk[lh, pass] = [128, 1024]: [a-chunks ic0..3 | b-chunks ic0..3]
    wqk = np.zeros((4, 2, 128, 1024), f32)
    for pidx, (wt, pr_a, pr_b) in enumerate((
            ("q", ((1, "r"), (1, "i")), ((-1, "i"), (1, "r"))),
            ("k", ((1, "r"), (-1, "i")), ((-1, "i"), (-1, "r"))))):
        WT_r, WT_i = (WqT_r, WqT_i) if wt == "q" else (WkT_r, WkT_i)
        src_ = {"r": WT_r, "i": WT_i}
        for lh in range(4):
            g = half * 4 + lh
            hs = slice(g * D, (g + 1) * D)
            a = np.concatenate([s * src_[k][:, hs] for s, k in pr_a], axis=1)
            bm = np.concatenate([s * src_[k][:, hs] for s, k in pr_b], axis=1)
            wqk[lh, pidx, :, 0:512] = _pack_ic(a).reshape(128, 512)
            wqk[lh, pidx, :, 512:1024] = _pack_ic(bm).reshape(128, 512)

    wv_a = headcols(WvT_r, WvT_i, ((1, "r"), (1, "i")))
    wv_b = headcols(WvT_r, WvT_i, ((-1, "i"), (1, "r")))
    wv_pack = np.stack([_pack_ic(wv_a), _pack_ic(wv_b)])

    wo_pack = np.zeros((128, 4096), f32)
    pack = np.zeros((1, 2560), np.float16)
    pack[0, 0:512] = 1.0
    for lh in range(4):
        g = half * 4 + lh
        hs = slice(g * D, (g + 1) * D)
        wo_pack[:, lh * 512:(lh + 1) * 512] = np.concatenate(
            [WoT_r[hs, :], -WoT_i[hs, :]], axis=0)
        wo_pack[:, 2048 + lh * 512:2048 + (lh + 1) * 512] = np.concatenate(
            [WoT_i[hs, :], WoT_r[hs, :]], axis=0)
        pack[0, 512 + lh * 128:512 + (lh + 1) * 128] = np.concatenate(
            [bv.real[hs], bv.imag[hs]])
        pack[0, 1024 + lh * 128:1024 + (lh + 1) * 128] = np.concatenate(
            [-bv.imag[hs], bv.real[hs]])
        pack[0, 1536 + lh * 128:1536 + (lh + 1) * 128] = np.concatenate(
            [bq.real[hs], bq.imag[hs]])
        pack[0, 2048 + lh * 128:2048 + (lh + 1) * 128] = np.concatenate(
            [bk.real[hs], -bk.imag[hs]])

    return {
        "xt_r": xt_r, "xt_i": xt_i,
        "wqk": wqk,
        "wv_pack": wv_pack,
        "wo_pack": wo_pack,
        "bias_pack": pack,
    }


def _enable_profiling():
    import sys
    import types
    if "antenv.axon_hooks" not in sys.modules:
        mod = types.ModuleType("antenv.axon_hooks")
        mod._hook = None
        mod.set_axon_ntff_profile_hook = lambda h: setattr(mod, "_hook", h)
        mod.get_axon_ntff_profile_hook = lambda: mod._hook
        sys.modules["antenv.axon_hooks"] = mod
        import antenv
        antenv.axon_hooks = mod
    from trn_agent_boot.trn_boot import _ntff_profile_via_ctypes
    sys.modules["antenv.axon_hooks"].set_axon_ntff_profile_hook(
        _ntff_profile_via_ctypes("/opt/axon/libaxon_pjrt.so"))
    import concourse.bass_utils as bu
    bu.upload_artifacts = lambda tmpdir: f"file://{tmpdir}"


def kernel(query, Wq, bq, Wk, bk, Wv, bv, Wo, bo):
    global last_exec_ns, last_scope_times
    from concourse.bass_utils import run_bass_kernel_spmd

    trace = os.environ.get("TRN_MHA_TRACE", "") == "1"
    if trace:
        _enable_profiling()

    if "nc" not in _prog_cache:
        _prog_cache["nc"] = _build_program()
    nc = _prog_cache["nc"]

    in_maps = []
    for c in range(N_CORES):
        b, half = c // 2, c % 2
        in_maps.append(_core_inputs(query, Wq, bq, Wk, bk, Wv, bv, Wo, bo,
                                    b, half))

    res = run_bass_kernel_spmd(nc, in_maps, list(range(N_CORES)), trace=trace)
    _prog_cache["last_res"] = res
    if trace:
        last_exec_ns = res.exec_time_ns
        last_scope_times = res.per_core_scope_times

    attn_output = np.zeros((B, T, C), np.complex64)
    attn_weights = np.zeros((B, H, T, T), np.complex64)
    for c in range(N_CORES):
        b, half = c // 2, c % 2
        r = res.results[c]
        attn_weights[b, half * 4:(half + 1) * 4] = r["attw_r"] + 1j * r["attw_i"]
        attn_output[b] += r["yp_r"] + 1j * r["yp_i"]
    attn_output += bo.astype(np.complex64)
    return (attn_output.astype(np.complex64),
            attn_weights.astype(np.complex64))


# revision 15
# speedup vs baseline: 1.0050x; 1.0050x over previous
"""Complex-valued multihead attention (B=4, T=1024, C=512, H=8) on 8 Trainium2
NeuronCores.

Sharding: core c = (b = c//2, half = c%2) handles batch b and heads
half*4 .. half*4+3 (tensor-parallel over heads within a batch).  The out_proj
is computed as per-core partials over each core's 256 head-dims and summed on
the host (the reduce of the head-TP all-gather), where the bo bias is added.

On-device math uses float32r (full-rate fp32 PE path, ~1e-4 component error).
Complex matmuls are computed as single K=128 matmuls by stacking real/imag
parts along the contraction axis, e.g.
  Sr = [Qr;Qi]^T-stationary x [Kr;-Ki]-moving  (t x s tile in one matmul).
Linear-layer biases are folded in as K=1 fp16 matmuls accumulating into PSUM.
"""
import os

import numpy as np

B, T, C, H = 4, 1024, 512, 8
D = C // H  # 64
O = 256     # head-dims per core (4 heads x 64)
SCALE = D ** (-0.5)
N_CORES = 8
IC = C // 128  # 4 contraction chunks over c_in
TP = T // 128  # 8 t-partition chunks
SF = T // 512  # 2 free-dim chunks of 512

_prog_cache = {}
last_exec_ns = None
last_scope_times = None


def _build_program():
    import concourse.bacc as bacc
    import concourse.tile as tile
    from concourse import mybir

    F32 = mybir.dt.float32
    F32R = mybir.dt.float32r
    F16 = mybir.dt.float16
    BF16 = mybir.dt.bfloat16
    AF = mybir.ActivationFunctionType
    ALU = mybir.AluOpType

    nc = bacc.Bacc("TRN2", target_bir_lowering=False, debug=False,
                   num_devices=N_CORES)

    # ---- DRAM I/O ----
    # xt: ic-major free-dim packing, one DMA each: [128, IC*T]
    xt_r = nc.dram_tensor("xt_r", [128, IC * T], F32R, kind="ExternalInput").ap()
    xt_i = nc.dram_tensor("xt_i", [128, IC * T], F32R, kind="ExternalInput").ap()
    # Q/K weights: per (head, pass): [128, 1024] = [a ic0..3 | b ic0..3]
    wqk = nc.dram_tensor("wqk", [4, 2, 128, 1024], F32R,
                         kind="ExternalInput").ap()
    # V weights: [2, 128, IC*512] = (wv1_a, wv1_b), ic-major packing
    wv_pack = nc.dram_tensor("wv_pack", [2, 128, IC * 512], F32R,
                             kind="ExternalInput").ap()
    # out-proj: [128, 4096] = [wo_r heads 0..3 | wo_i heads 0..3]
    wo_pack = nc.dram_tensor("wo_pack", [128, 4096], F32R,
                             kind="ExternalInput").ap()
    # col blocks: [0:512]=ones, [512:1024]=bias_va, [1024:1536]=bias_vb,
    # then 8 blocks of 128: bq[lh]*4, bk1[lh]*4
    bias_pack = nc.dram_tensor("bias_pack", [1, 2560], F16,
                               kind="ExternalInput").ap()

    attw_r = nc.dram_tensor("attw_r", [4, T, T], F32, kind="ExternalOutput").ap()
    attw_i = nc.dram_tensor("attw_i", [4, T, T], F32, kind="ExternalOutput").ap()
    yp_r = nc.dram_tensor("yp_r", [T, C], F32, kind="ExternalOutput").ap()
    yp_i = nc.dram_tensor("yp_i", [T, C], F32, kind="ExternalOutput").ap()

    with tile.TileContext(nc) as tc:
        with (
            tc.tile_pool(name="xt", bufs=1) as xt_pool,
            tc.tile_pool(name="wt", bufs=2) as wt_pool,
            tc.tile_pool(name="wv", bufs=1) as wv_pool,
            tc.tile_pool(name="wo", bufs=1) as wo_pool,
            tc.tile_pool(name="qk", bufs=2) as qk_pool,
            tc.tile_pool(name="vs", bufs=1) as vs_pool,
            tc.tile_pool(name="st", bufs=16) as st_pool,
            tc.tile_pool(name="sdr", bufs=3) as sdr_pool,
            tc.tile_pool(name="av", bufs=1) as av_pool,
            tc.tile_pool(name="yp", bufs=2) as yp_pool,
            tc.tile_pool(name="cst", bufs=1) as cst_pool,
            tc.tile_pool(name="pp", bufs=2, space="PSUM") as pp_pool,
            tc.tile_pool(name="psc", bufs=5, space="PSUM") as psc_pool,
            tc.tile_pool(name="pav", bufs=1, space="PSUM") as pav_pool,
        ):
            # ---- inputs: large DMAs split in halves across three queues,
            # ordered so the V-projection's operands land first ----
            xtr_all = xt_pool.tile([128, IC * T], F32R, tag="xtr")
            half_x = IC * T // 2
            nc.sync.dma_start(xtr_all[:, 0:half_x], xt_r[:, 0:half_x])
            xti_all = xt_pool.tile([128, IC * T], F32R, tag="xti")
            nc.gpsimd.dma_start(xti_all[:, 0:half_x], xt_i[:, 0:half_x])
            wv_sb = []
            for v in range(2):
                t = wv_pool.tile([128, IC * 512], F32R, tag=f"wv{v}")
                nc.scalar.dma_start(t[:], wv_pack[v])
                wv_sb.append(t)
            nc.sync.dma_start(xtr_all[:, half_x:], xt_r[:, half_x:])
            nc.gpsimd.dma_start(xti_all[:, half_x:], xt_i[:, half_x:])
            xtr = [xtr_all[:, ic * T:(ic + 1) * T] for ic in range(IC)]
            xti = [xti_all[:, ic * T:(ic + 1) * T] for ic in range(IC)]

            bias_all = cst_pool.tile([1, 2560], F16, tag="bias_all")
            nc.scalar.dma_start(bias_all[:], bias_pack[:])
            ones_sb = bias_all[:, 0:512]
            bva_sb = bias_all[:, 512:1024]
            bvb_sb = bias_all[:, 1024:1536]
            bias_sb = {}
            for j, nm in enumerate(("bq", "bk1")):
                for lh in range(4):
                    off = 1536 + (j * 4 + lh) * 128
                    bias_sb[(nm, lh)] = bias_all[:, off:off + 128]

            wo_all = wo_pool.tile([128, 4096], F32R, tag="wo")
            nc.scalar.dma_start(wo_all[:], wo_pack[:])
            wo_sb = {}
            for lh in range(4):
                wo_sb[("r", lh)] = wo_all[:, lh * 512:(lh + 1) * 512]
                wo_sb[("i", lh)] = wo_all[:, 2048 + lh * 512:2048 + (lh + 1) * 512]

            # ---- vstk_a projection (all heads batched); vstk_b derived ----
            # vstk_a = [Vr_h|Vi_h]x4 heads; vstk_b = [-Vi_h|Vr_h]x4 via ACT
            # column swap + negate from vstk_a.
            vstk = {}
            scope_vproj = nc.named_scope("vproj")
            scope_vproj.__enter__()
            for tp in range(TP):
                ps = pp_pool.tile([128, 512], F32, tag="pp")
                for ic in range(IC):
                    nc.tensor.matmul(
                        ps[:], xtr[ic][:, tp * 128:(tp + 1) * 128],
                        wv_sb[0][:, ic * 512:(ic + 1) * 512],
                        start=(ic == 0), stop=False)
                    nc.tensor.matmul(
                        ps[:], xti[ic][:, tp * 128:(tp + 1) * 128],
                        wv_sb[1][:, ic * 512:(ic + 1) * 512],
                        start=False, stop=False)
                nc.tensor.matmul(
                    ps[:], ones_sb[:, 0:128], bva_sb,
                    start=False, stop=True)
                dst = vs_pool.tile([128, 512], F32R, tag=f"va_{tp}")
                nc.scalar.activation(dst[:], ps[:], AF.Identity,
                                     bias=0.0, scale=1.0)
                vstk[("a", tp)] = dst
                dst_b = vs_pool.tile([128, 512], F32R, tag=f"vb_{tp}")
                for lh in range(4):
                    o = lh * 128
                    nc.scalar.activation(dst_b[:, o:o + 64],
                                         dst[:, o + 64:o + 128],
                                         AF.Identity, bias=0.0, scale=-1.0)
                    nc.scalar.activation(dst_b[:, o + 64:o + 128],
                                         dst[:, o:o + 64],
                                         AF.Identity, bias=0.0, scale=1.0)
                vstk[("b", tp)] = dst_b

            scope_vproj.__exit__(None, None, None)
            avstk = {}
            qk_state = {}

            def emit_proj(lh, name, bnm, pidx):
                w_t = wt_pool.tile([128, 1024], F32R, tag="w")
                (nc.sync if pidx == 0 else nc.gpsimd).dma_start(
                    w_t[:], wqk[lh, pidx])
                dst = qk_pool.tile([128, T], F32R, tag=name)
                for tf in range(SF):
                    ps = pp_pool.tile([128, 512], F32, tag="pp")
                    for ic in range(IC):
                        nc.tensor.matmul(
                            ps[:], w_t[:, ic * 128:(ic + 1) * 128],
                            xtr[ic][:, tf * 512:(tf + 1) * 512],
                            start=(ic == 0), stop=False)
                        nc.tensor.matmul(
                            ps[:], w_t[:, 512 + ic * 128:512 + (ic + 1) * 128],
                            xti[ic][:, tf * 512:(tf + 1) * 512],
                            start=False, stop=False)
                    nc.tensor.matmul(
                        ps[:], bias_sb[(bnm, lh)], ones_sb,
                        start=False, stop=True)
                    nc.scalar.activation(dst[:, tf * 512:(tf + 1) * 512],
                                         ps[:], AF.Identity,
                                         bias=0.0, scale=1.0)
                qk_state[(lh, name)] = dst

            def emit_kts(lh):
                ktn = qk_state[(lh, "ktn")]
                kts = qk_pool.tile([128, T], F32R, tag="kts")
                nc.gpsimd.dma_start(kts[0:64, :], ktn[64:128, :])
                nc.sync.dma_start(kts[64:128, :], ktn[0:64, :])
                nc.scalar.activation(kts[0:64, :], kts[0:64, :], AF.Identity,
                                     bias=0.0, scale=-1.0)
                qk_state[(lh, "kts")] = kts

            def emit_scores(lh, th, part, stt):
                qt = qk_state[(lh, "qt")]
                kmat = qk_state[(lh, "ktn" if part == "r" else "kts")]
                for k in range(8):
                    sp = k
                    ps_st = psc_pool.tile([128, 512], F32, tag="psc")
                    nc.tensor.matmul(
                        ps_st[:], kmat[:, sp * 128:(sp + 1) * 128],
                        qt[:, th * 512:(th + 1) * 512],
                        start=True, stop=True)
                    st_t = st_pool.tile([128, 512], F32R, tag="st")
                    nc.scalar.activation(st_t[:], ps_st[:], AF.Relu,
                                         bias=0.0, scale=SCALE)
                    stt[(part, sp)] = st_t

                    tp = th * 4 + k // 2
                    sf = k % 2
                    ps_s = psc_pool.tile([128, 512], F32, tag="psc")
                    nc.tensor.matmul(
                        ps_s[:], qt[:, tp * 128:(tp + 1) * 128],
                        kmat[:, sf * 512:(sf + 1) * 512],
                        start=True, stop=True)
                    s_t = sdr_pool.tile([128, 512], F32, tag="sdr")
                    nc.vector.tensor_scalar(s_t[:], ps_s[:], SCALE,
                                            0.0, ALU.mult, ALU.max)
                    attw = attw_r if part == "r" else attw_i
                    eng = nc.sync if part == "r" else nc.gpsimd
                    eng.dma_start(
                        attw[lh, tp * 128:(tp + 1) * 128,
                             sf * 512:(sf + 1) * 512], s_t[:])

            def emit_av(lh, th, stt):
                ps_av = pav_pool.tile([128, 512], F32, tag="pav")
                for sp in range(8):
                    nc.tensor.matmul(
                        ps_av[:],
                        vstk[("a", sp)][:, lh * 128:(lh + 1) * 128],
                        stt[("r", sp)][:],
                        start=(sp == 0), stop=False)
                    nc.tensor.matmul(
                        ps_av[:],
                        vstk[("b", sp)][:, lh * 128:(lh + 1) * 128],
                        stt[("i", sp)][:],
                        start=False, stop=(sp == 7))
                if th == 0:
                    av_sb = av_pool.tile([128, T], F32R, tag=f"av{lh}")
                    avstk[lh] = av_sb
                nc.scalar.activation(
                    avstk[lh][:, th * 512:(th + 1) * 512], ps_av[:],
                    AF.Identity, bias=0.0, scale=1.0)

            def emit_yp(tp):
                ps_r = pp_pool.tile([128, 512], F32, tag="pp")
                ps_i = pp_pool.tile([128, 512], F32, tag="pp")
                for lh in range(4):
                    nc.tensor.matmul(ps_r[:],
                                     avstk[lh][:, tp * 128:(tp + 1) * 128],
                                     wo_sb[("r", lh)][:],
                                     start=(lh == 0), stop=(lh == 3))
                for lh in range(4):
                    nc.tensor.matmul(ps_i[:],
                                     avstk[lh][:, tp * 128:(tp + 1) * 128],
                                     wo_sb[("i", lh)][:],
                                     start=(lh == 0), stop=(lh == 3))
                o_r = yp_pool.tile([128, 512], F32, tag="yp")
                nc.vector.tensor_copy(o_r[:], ps_r[:])
                nc.sync.dma_start(yp_r[tp * 128:(tp + 1) * 128, :], o_r[:])
                o_i = yp_pool.tile([128, 512], F32, tag="yp")
                nc.vector.tensor_copy(o_i[:], ps_i[:])
                nc.gpsimd.dma_start(yp_i[tp * 128:(tp + 1) * 128, :], o_i[:])

            # prologue: head0 projections
            emit_proj(0, "qt", "bq", 0)
            emit_proj(0, "ktn", "bk1", 1)
            emit_kts(0)
            for lh in range(4):
                scope_h = nc.named_scope(f"head{lh}")
                scope_h.__enter__()
                stt0 = {}
                emit_scores(lh, 0, "r", stt0)
                emit_scores(lh, 0, "i", stt0)
                if lh < 3:
                    emit_proj(lh + 1, "qt", "bq", 0)
                emit_av(lh, 0, stt0)
                if lh < 3:
                    emit_proj(lh + 1, "ktn", "bk1", 1)
                    emit_kts(lh + 1)
                stt1 = {}
                emit_scores(lh, 1, "r", stt1)
                emit_scores(lh, 1, "i", stt1)
                if lh == 3:
                    for tp in range(4):
                        emit_yp(tp)
                emit_av(lh, 1, stt1)
                scope_h.__exit__(None, None, None)
            # ---- out_proj partials ----
            scope_yp = nc.named_scope("yp")
            scope_yp.__enter__()
            for tp in range(4, TP):
                emit_yp(tp)

            scope_yp.__exit__(None, None, None)
    nc.compile()
    return nc


def _head_stacks(Wr, Wi, hs):
    # -> per-head (512, 128) transposed stationary blocks
    return Wr[hs, :].T.copy(), Wi[hs, :].T.copy()


def _as_chunks(m):
    # (512, 128) -> (IC, 128, 128)
    return np.ascontiguousarray(m.reshape(IC, 128, 128))


def _pack_ic(m):
    # (512, N) -> (128, IC*N): ic-major packing of row chunks into free dim
    n = m.shape[1]
    return np.ascontiguousarray(
        m.reshape(IC, 128, n).transpose(1, 0, 2).reshape(128, IC * n))


def _core_inputs(query, Wq, bq, Wk, bk, Wv, bv, Wo, bo, b, half):
    f32 = np.float32
    x = query[b]
    xt_r = _pack_ic(np.ascontiguousarray(x.real.T).astype(f32))
    xt_i = _pack_ic(np.ascontiguousarray(x.imag.T).astype(f32))

    WqT_r, WqT_i = Wq.real.T.astype(f32), Wq.imag.T.astype(f32)
    WkT_r, WkT_i = Wk.real.T.astype(f32), Wk.imag.T.astype(f32)
    WvT_r, WvT_i = Wv.real.T.astype(f32), Wv.imag.T.astype(f32)
    WoT_r, WoT_i = Wo.real.T.astype(f32), Wo.imag.T.astype(f32)

    def headcols(WT_r, WT_i, pair):
        # (512, 128) per head: [left 64 | right 64] col blocks, then all
        # heads side by side -> (512, 512)
        src_ = {"r": WT_r, "i": WT_i}
        cols = []
        for lh in range(4):
            g = half * 4 + lh
            hs = slice(g * D, (g + 1) * D)
            cols.append(np.concatenate(
                [s * src_[k][:, hs] for s, k in pair], axis=1))
        return np.concatenate(cols, axis=1)

    # wqk[lh, pass] = [128, 1024]: [a-chunks ic0..3 | b-chunks ic0..3]
    wqk = np.zeros((4, 2, 128, 1024), f32)
    for pidx, (wt, pr_a, pr_b) in enumerate((
            ("q", ((1, "r"), (1, "i")), ((-1, "i"), (1, "r"))),
            ("k", ((1, "r"), (-1, "i")), ((-1, "i"), (-1, "r"))))):
        WT_r, WT_i = (WqT_r, WqT_i) if wt == "q" else (WkT_r, WkT_i)
        src_ = {"r": WT_r, "i": WT_i}
        for lh in range(4):
            g = half * 4 + lh
            hs = slice(g * D, (g + 1) * D)
            a = np.concatenate([s * src_[k][:, hs] for s, k in pr_a], axis=1)
            bm = np.concatenate([s * src_[k][:, hs] for s, k in pr_b], axis=1)
            wqk[lh, pidx, :, 0:512] = _pack_ic(a).reshape(128, 512)
            wqk[lh, pidx, :, 512:1024] = _pack_ic(bm).reshape(128, 512)

    wv_a = headcols(WvT_r, WvT_i, ((1, "r"), (1, "i")))
    wv_b = headcols(WvT_r, WvT_i, ((-1, "i"), (1, "r")))
    wv_pack = np.stack([_pack_ic(wv_a), _pack_ic(wv_b)])

    wo_pack = np.zeros((128, 4096), f32)
    pack = np.zeros((1, 2560), np.float16)
    pack[0, 0:512] = 1.0
    for lh in range(4):
        g = half * 4 + lh
        hs = slice(g * D, (g + 1) * D)
        wo_pack[:, lh * 512:(lh + 1) * 512] = np.concatenate(
            [WoT_r[hs, :], -WoT_i[hs, :]], axis=0)
        wo_pack[:, 2048 + lh * 512:2048 + (lh + 1) * 512] = np.concatenate(
            [WoT_i[hs, :], WoT_r[hs, :]], axis=0)
        pack[0, 512 + lh * 128:512 + (lh + 1) * 128] = np.concatenate(
            [bv.real[hs], bv.imag[hs]])
        pack[0, 1024 + lh * 128:1024 + (lh + 1) * 128] = np.concatenate(
            [-bv.imag[hs], bv.real[hs]])
        pack[0, 1536 + lh * 128:1536 + (lh + 1) * 128] = np.concatenate(
            [bq.real[hs], bq.imag[hs]])
        pack[0, 2048 + lh * 128:2048 + (lh + 1) * 128] = np.concatenate(
            [bk.real[hs], -bk.imag[hs]])

    return {
        "xt_r": xt_r, "xt_i": xt_i,
        "wqk": wqk,
        "wv_pack": wv_pack,
        "wo_pack": wo_pack,
        "bias_pack": pack,
    }


def _enable_profiling():
    import sys
    import types
    if "antenv.axon_hooks" not in sys.modules:
        mod = types.ModuleType("antenv.axon_hooks")
        mod._hook = None
        mod.set_axon_ntff_profile_hook = lambda h: setattr(mod, "_hook", h)
        mod.get_axon_ntff_profile_hook = lambda: mod._hook
        sys.modules["antenv.axon_hooks"] = mod
        import antenv
        antenv.axon_hooks = mod
    from trn_agent_boot.trn_boot import _ntff_profile_via_ctypes
    sys.modules["antenv.axon_hooks"].set_axon_ntff_profile_hook(
        _ntff_profile_via_ctypes("/opt/axon/libaxon_pjrt.so"))
    import concourse.bass_utils as bu
    bu.upload_artifacts = lambda tmpdir: f"file://{tmpdir}"


def kernel(query, Wq, bq, Wk, bk, Wv, bv, Wo, bo):
    global last_exec_ns, last_scope_times
    from concourse.bass_utils import run_bass_kernel_spmd

    trace = os.environ.get("TRN_MHA_TRACE", "") == "1"
    if trace:
        _enable_profiling()

    if "nc" not in _prog_cache:
        _prog_cache["nc"] = _build_program()
    nc = _prog_cache["nc"]

    in_maps = []
    for c in range(N_CORES):
        b, half = c // 2, c % 2
        in_maps.append(_core_inputs(query, Wq, bq, Wk, bk, Wv, bv, Wo, bo,
                                    b, half))

    res = run_bass_kernel_spmd(nc, in_maps, list(range(N_CORES)), trace=trace)
    _prog_cache["last_res"] = res
    if trace:
        last_exec_ns = res.exec_time_ns
        last_scope_times = res.per_core_scope_times

    attn_output = np.zeros((B, T, C), np.complex64)
    attn_weights = np.zeros((B, H, T, T), np.complex64)
    for c in range(N_CORES):
        b, half = c // 2, c % 2
        r = res.results[c]
        attn_weights[b, half * 4:(half + 1) * 4] = r["attw_r"] + 1j * r["attw_i"]
        attn_output[b] += r["yp_r"] + 1j * r["yp_i"]
    attn_output += bo.astype(np.complex64)
    return (attn_output.astype(np.complex64),
            attn_weights.astype(np.complex64))
